# revision 1
# baseline (speedup 1.0000x reference)
"""AFNO2D block (Hartley-transform spectral MLP) on 8 TRN2 NeuronCores.

Strategy
--------
The reference contracts only the W and C axes (the "2D" DHT is over width and
channels); H is embarrassingly parallel.  The spectral-negation partner of row
h is row (H-h)%H, so rows are assigned to cores in (h, ph) pairs.  With the
host pre-reversing the partner row along W (xb = rev_w(x[ph])), the even/odd
split u = xa+xb, z = xa-xb makes E = DHT(u)/2-ish, O = DHT(z)/2 and the whole
spectral MLP + reversal algebra reduces to plain per-column matmuls: all index
reversals commute out to the host (input pre-reverse, output un-reverse).

Per pair-slot (2 rows) on-device:
  u,z -> W-DHT (matmul) -> PE-transpose -> C-DHT (matmul) -> E,O
  P+Q / P-Q banks from E@(w1[0]/2), O@(+-w1[1]/2) accumulated in PSUM
  A1=relu(P+Q+b1k) B1=relu(P-Q+b1n) A2=relu(P-Q+b1k) B2=relu(P+Q+b1n)
  the o2k stage is algebraically folded into the final matmuls:
  D = A@(w2a@w2bI) + B@(w2a + w2b@w2bI),  y = D + (b2k@w2bI + b2n)
  s = softshrink(y) = y - clamp(y, -l, l)
  C-DHT -> PE-transpose -> W-DHT(scaled 1/(W*C)) -> out (= correction only;
  the +x residual is added on the host during unshard, in full fp32)

Matmuls/transposes run in bf16 (fp32 PSUM accumulate; the residual is added
host-side in fp32, so overall rel err ~1.5e-4).  8 cores x 12 slots = 192
row-positions for 180 rows (+ self-paired rows 0/90 occupy full slots, rest
zero padding).  No collectives; each core is fully independent.
"""

import numpy as np

import ml_dtypes

BF16 = ml_dtypes.bfloat16

H, W, C = 180, 360, 512
NB, BS = 8, 64
LAM = 0.01
PADW = 384          # W padded to 3 chunks of 128 on the contraction side
NSLOT = 12          # pair-slots per core
RPC = 2 * NSLOT     # row-positions per core
NCORES = 8

# w / v chunking of the 360-sized axis
WCH = [(0, 128), (128, 128), (256, 104)]
NDC = 4             # 512 = 4 chunks of 128 (c and d axes)

_NC = None          # cached Bass graph


def _cas(n):
    t = np.arange(n, dtype=np.float64)
    a = 2.0 * np.pi * np.outer(t, t) / n
    return (np.cos(a) + np.sin(a)).astype(np.float32)


def _revw(row):
    # row: (W, C) -> row'[w] = row[(-w) % W]
    return np.roll(row[::-1], 1, axis=0)


def _slots():
    s = [(h, (H - h) % H) for h in range(1, H // 2)]      # 89 pairs
    s += [(0, 0), (90, 90)]                                # self-paired
    s += [None] * (NCORES * NSLOT - len(s))                # padding
    return s


def _blockdiag(m):
    # m: (8, 64, 64) -> (4, 128, 128) block-diagonal pairs
    out = np.zeros((NDC, 128, 128), dtype=np.float32)
    for j in range(NDC):
        out[j, :64, :64] = m[2 * j]
        out[j, 64:, 64:] = m[2 * j + 1]
    return out


def _build_nc():
    from contextlib import ExitStack

    import concourse.bass as bass
    import concourse.mybir as mybir
    import concourse.tile as tile
    from concourse import bacc

    f32 = mybir.dt.float32
    bf16 = mybir.dt.bfloat16
    ADD = mybir.AluOpType.add
    MAX = mybir.AluOpType.max
    MIN = mybir.AluOpType.min
    RELU = mybir.ActivationFunctionType.Relu
    IDENT = mybir.ActivationFunctionType.Identity

    nc = bacc.Bacc()
    x_ext = nc.declare_dram_parameter("x", [RPC, PADW, C], f32, isOutput=False)
    casc_ext = nc.declare_dram_parameter("casc", [C, C], bf16, isOutput=False)
    caswf_ext = nc.declare_dram_parameter("caswf", [PADW, W], bf16, isOutput=False)
    caswi_ext = nc.declare_dram_parameter("caswi", [PADW, W], bf16, isOutput=False)
    mlpw_ext = nc.declare_dram_parameter("mlpw", [6, NDC, 128, 128], bf16, isOutput=False)
    bias_ext = nc.declare_dram_parameter("biases", [128, 5, NDC], f32, isOutput=False)
    id_ext = nc.declare_dram_parameter("ident", [128, 128], bf16, isOutput=False)
    out_ext = nc.declare_dram_parameter("out", [RPC, W, C], f32, isOutput=True)

    with tile.TileContext(nc) as tc, ExitStack() as ctx:
        consts = ctx.enter_context(tc.tile_pool(name="consts", bufs=1))
        casc = consts.tile([128, NDC, C], bf16)
        nc.sync.dma_start(out=casc, in_=casc_ext[:, :].rearrange("(a p) d -> p a d", p=128))
        caswf = consts.tile([128, 3, W], bf16)
        nc.sync.dma_start(out=caswf, in_=caswf_ext[:, :].rearrange("(k p) v -> p k v", p=128))
        caswi = consts.tile([128, 3, W], bf16)
        nc.sync.dma_start(out=caswi, in_=caswi_ext[:, :].rearrange("(k p) v -> p k v", p=128))
        mlpw = consts.tile([128, 6, NDC, 128], bf16)
        nc.sync.dma_start(out=mlpw, in_=mlpw_ext[:, :, :, :].rearrange("s j p o -> p s j o"))
        biases = consts.tile([128, 5, NDC], f32)
        nc.sync.dma_start(out=biases, in_=bias_ext[:, :, :])
        ident = consts.tile([128, 128], bf16)
        nc.sync.dma_start(out=ident, in_=id_ext[:, :])

        xin = ctx.enter_context(tc.tile_pool(name="xin", bufs=4))
        uzp = ctx.enter_context(tc.tile_pool(name="uzp", bufs=4))
        xhw = ctx.enter_context(tc.tile_pool(name="xhw", bufs=4))
        xht = ctx.enter_context(tc.tile_pool(name="xht", bufs=4))
        eo = ctx.enter_context(tc.tile_pool(name="eo", bufs=4))
        sml = ctx.enter_context(tc.tile_pool(name="sml", bufs=16))
        s12 = ctx.enter_context(tc.tile_pool(name="s12", bufs=4))
        scp = ctx.enter_context(tc.tile_pool(name="scp", bufs=4))
        stp = ctx.enter_context(tc.tile_pool(name="stp", bufs=4))
        outp = ctx.enter_context(tc.tile_pool(name="outp", bufs=8))
        psmm = ctx.enter_context(tc.tile_pool(name="psmm", bufs=6, space="PSUM"))
        pstp = ctx.enter_context(tc.tile_pool(name="pstp", bufs=2, space="PSUM"))

        def b_ap(which, dc):
            return biases[:, which, dc : dc + 1]

        for s in range(NSLOT):
            # ---- load the pair, form even/odd combinations -------------
            xah = xin.tile([128, 3, C], bf16, tag="xinh")
            nc.gpsimd.dma_start(out=xah, in_=x_ext[2 * s].rearrange("(k p) c -> p k c", p=128))
            xbh = xin.tile([128, 3, C], bf16, tag="xinh")
            nc.gpsimd.dma_start(out=xbh, in_=x_ext[2 * s + 1].rearrange("(k p) c -> p k c", p=128))
            u = uzp.tile([128, 3, C], bf16, tag="uz")
            z = uzp.tile([128, 3, C], bf16, tag="uz")
            nc.vector.tensor_add(u, xah, xbh)
            nc.vector.tensor_sub(z, xah, xbh)

            # ---- forward W-transform:  (w,c) -> (v,c) ------------------
            hw_u = xhw.tile([128, 3, C], bf16, tag="xhw")
            hw_z = xhw.tile([128, 3, C], bf16, tag="xhw")
            for t, dst in ((u, hw_u), (z, hw_z)):
                for vc, (voff, vsz) in enumerate(WCH):
                    ps = psmm.tile([128, 512], f32, tag="mm")
                    for wc in range(3):
                        nc.tensor.matmul(
                            ps[:vsz],
                            lhsT=caswf[:, wc, voff : voff + vsz],
                            rhs=t[:, wc, :],
                            start=(wc == 0),
                            stop=(wc == 2),
                        )
                    nc.scalar.copy(dst[:vsz, vc, :], ps[:vsz])

            # ---- transpose to (c,v) ------------------------------------
            ht_u = xht.tile([128, NDC, W], bf16, tag="xht")
            ht_z = xht.tile([128, NDC, W], bf16, tag="xht")
            for t, dst in ((hw_u, ht_u), (hw_z, ht_z)):
                for cc in range(NDC):
                    pst = pstp.tile([128, 512], bf16, tag="tp")
                    for vc, (voff, vsz) in enumerate(WCH):
                        nc.tensor.transpose(
                            pst[:, voff : voff + vsz],
                            in_=t[:vsz, vc, cc * 128 : (cc + 1) * 128],
                            identity=ident[:vsz, :vsz],
                        )
                    nc.vector.tensor_copy(dst[:, cc, :], pst[:, :W])

            # ---- forward C-transform: (c,v) -> (d,v) => E, O -----------
            E = eo.tile([128, NDC, W], bf16, tag="eo")
            O = eo.tile([128, NDC, W], bf16, tag="eo")
            for dc in range(NDC):
                ps_e = psmm.tile([128, 512], f32, tag="mm")
                ps_o = psmm.tile([128, 512], f32, tag="mm")
                for cc in range(NDC):
                    for t, ps in ((ht_u, ps_e), (ht_z, ps_o)):
                        nc.tensor.matmul(
                            ps[:, :W],
                            lhsT=casc[:, cc, dc * 128 : (dc + 1) * 128],
                            rhs=t[:, cc, :],
                            start=(cc == 0),
                            stop=(cc == NDC - 1),
                        )
                nc.scalar.copy(E[:, dc, :], ps_e[:, :W])
                nc.scalar.copy(O[:, dc, :], ps_o[:, :W])

            # ---- spectral MLP ------------------------------------------
            sh1 = s12.tile([128, NDC, W], bf16, tag="s12")
            sh2 = s12.tile([128, NDC, W], bf16, tag="s12")
            for dc in range(NDC):
                pq = psmm.tile([128, 512], f32, tag="mm")
                nc.tensor.matmul(pq[:, :W], lhsT=mlpw[:, 0, dc, :], rhs=E[:, dc, :], start=True, stop=False)
                nc.tensor.matmul(pq[:, :W], lhsT=mlpw[:, 1, dc, :], rhs=O[:, dc, :], start=False, stop=True)
                pmq = psmm.tile([128, 512], f32, tag="mm")
                nc.tensor.matmul(pmq[:, :W], lhsT=mlpw[:, 0, dc, :], rhs=E[:, dc, :], start=True, stop=False)
                nc.tensor.matmul(pmq[:, :W], lhsT=mlpw[:, 2, dc, :], rhs=O[:, dc, :], start=False, stop=True)

                A1 = sml.tile([128, W], bf16, tag="sml")
                B1 = sml.tile([128, W], bf16, tag="sml")
                A2 = sml.tile([128, W], bf16, tag="sml")
                B2 = sml.tile([128, W], bf16, tag="sml")
                nc.vector.tensor_scalar(A1, pq[:, :W], b_ap(0, dc), 0.0, op0=ADD, op1=MAX)
                nc.vector.tensor_scalar(B1, pmq[:, :W], b_ap(1, dc), 0.0, op0=ADD, op1=MAX)
                nc.scalar.activation(A2, pmq[:, :W], RELU, bias=b_ap(0, dc), scale=1.0)
                nc.scalar.activation(B2, pq[:, :W], RELU, bias=b_ap(1, dc), scale=1.0)

                for At, Bt, sh in ((A1, B1, sh1), (A2, B2, sh2)):
                    d = psmm.tile([128, 512], f32, tag="mm")
                    nc.tensor.matmul(d[:, :W], lhsT=mlpw[:, 3, dc, :], rhs=At, start=True, stop=False)
                    nc.tensor.matmul(d[:, :W], lhsT=mlpw[:, 4, dc, :], rhs=Bt, start=False, stop=True)
                    yt = sml.tile([128, W], bf16, tag="sml")
                    nc.vector.tensor_scalar_add(yt, d[:, :W], b_ap(3, dc))
                    cl = sml.tile([128, W], bf16, tag="sml")
                    nc.vector.tensor_scalar(cl, yt, -LAM, LAM, op0=MAX, op1=MIN)
                    nc.gpsimd.tensor_sub(sh[:, dc, :], yt, cl)

            # ---- inverse C-transform: (d,v) -> (c,v) -------------------
            sc1 = scp.tile([128, NDC, W], bf16, tag="sc")
            sc2 = scp.tile([128, NDC, W], bf16, tag="sc")
            for cc in range(NDC):
                ps_1 = psmm.tile([128, 512], f32, tag="mm")
                ps_2 = psmm.tile([128, 512], f32, tag="mm")
                for dc in range(NDC):
                    for t, ps in ((sh1, ps_1), (sh2, ps_2)):
                        nc.tensor.matmul(
                            ps[:, :W],
                            lhsT=casc[:, dc, cc * 128 : (cc + 1) * 128],
                            rhs=t[:, dc, :],
                            start=(dc == 0),
                            stop=(dc == NDC - 1),
                        )
                nc.scalar.copy(sc1[:, cc, :], ps_1[:, :W])
                nc.scalar.copy(sc2[:, cc, :], ps_2[:, :W])

            # ---- transpose back to (v,c) -------------------------------
            st1 = stp.tile([128, 3, C], bf16, tag="st")
            st2 = stp.tile([128, 3, C], bf16, tag="st")
            for t, dst in ((sc1, st1), (sc2, st2)):
                for vc, (voff, vsz) in enumerate(WCH):
                    pst = pstp.tile([128, 512], bf16, tag="tp")
                    for cc in range(NDC):
                        nc.tensor.transpose(
                            pst[:vsz, cc * 128 : (cc + 1) * 128],
                            in_=t[:, cc, voff : voff + vsz],
                            identity=ident,
                        )
                    nc.scalar.copy(dst[:vsz, vc, :], pst[:vsz])

            # ---- inverse W-transform + residual bias + store -----------
            for t, row in ((st1, 2 * s), (st2, 2 * s + 1)):
                for wc, (woff, wsz) in enumerate(WCH):
                    ps = psmm.tile([128, 512], f32, tag="mm")
                    for vc, (voff, vsz) in enumerate(WCH):
                        nc.tensor.matmul(
                            ps[:wsz],
                            lhsT=caswi[:vsz, vc, woff : woff + wsz],
                            rhs=t[:vsz, vc, :],
                            start=(vc == 0),
                            stop=(vc == 2),
                        )
                    ot = outp.tile([128, C], f32, tag="outp")
                    nc.vector.tensor_copy(ot[:wsz], ps[:wsz])
                    nc.sync.dma_start(out=out_ext[row, woff : woff + wsz, :], in_=ot[:wsz])

    nc.finalize()
    return nc


def _host_prep(x, w1, b1, w2, b2):
    x = np.asarray(x, dtype=np.float32).reshape(H, W, C)
    w1 = np.asarray(w1, dtype=np.float32)
    b1 = np.asarray(b1, dtype=np.float32)
    w2 = np.asarray(w2, dtype=np.float32)
    b2 = np.asarray(b2, dtype=np.float32)

    casc = _cas(C)
    casw = _cas(W)
    caswf = np.zeros((PADW, W), dtype=np.float32)
    caswf[:W] = casw
    caswi = np.zeros((PADW, W), dtype=np.float32)
    caswi[:W] = casw / np.float32(W * C)

    w2a = 0.5 * (w2[0] + w2[1])
    w2b = 0.5 * (w2[0] - w2[1])
    w2bi = w2b + np.eye(BS, dtype=np.float32)[None]
    g1 = np.einsum("kio,kop->kip", w2a, w2bi)
    g2 = w2a + np.einsum("kio,kop->kip", w2b, w2bi)
    mlpw = np.stack(
        [
            _blockdiag(0.5 * w1[0]),
            _blockdiag(0.5 * w1[1]),
            _blockdiag(-0.5 * w1[1]),
            _blockdiag(g1),
            _blockdiag(g2),
            _blockdiag(w2bi),
        ]
    ).astype(np.float32)

    b2ki = np.einsum("ki,kip->kp", b2[0], w2bi)
    bvecs = [
        b1[0].reshape(C),
        b1[1].reshape(C),
        b2[0].reshape(C),
        b2ki.reshape(C) + b2[1].reshape(C),
        -b2[1].reshape(C) - LAM,
    ]
    biases = np.zeros((128, 5, NDC), dtype=np.float32)
    for i, v in enumerate(bvecs):
        biases[:, i, :] = v.reshape(NDC, 128).T

    ident = np.eye(128, dtype=np.float32)

    slots = _slots()
    shards = []
    for c in range(NCORES):
        sh = np.zeros((RPC, PADW, C), dtype=np.float32)
        for si in range(NSLOT):
            slot = slots[c * NSLOT + si]
            if slot is None:
                continue
            a, b = slot
            sh[2 * si, :W] = x[a]
            sh[2 * si + 1, :W] = _revw(x[b])
        shards.append(sh)

    weights = {
        "casc": casc.astype(BF16),
        "caswf": caswf.astype(BF16),
        "caswi": caswi.astype(BF16),
        "mlpw": mlpw.astype(BF16),
        "biases": biases,
        "ident": ident.astype(BF16),
    }
    return shards, weights, slots


def _ensure_ntff_hook():
    """The agent image's ``antenv`` lacks ``axon_hooks``; provide a shim so
    ``run_bass_kernel_spmd(trace=True)`` can profile under axon."""
    try:
        from antenv import axon_hooks  # noqa: F401

        return True
    except ImportError:
        pass
    try:
        import sys
        import types

        import antenv
        from trn_agent_boot.trn_boot import _ntff_profile_via_ctypes

        mod = types.ModuleType("antenv.axon_hooks")
        state = {"hook": None}
        mod.set_axon_ntff_profile_hook = lambda h: state.__setitem__("hook", h)
        mod.get_axon_ntff_profile_hook = lambda: state["hook"]
        sys.modules["antenv.axon_hooks"] = mod
        antenv.axon_hooks = mod
        hook = _ntff_profile_via_ctypes("/opt/axon/libaxon_pjrt.so")
        mod.set_axon_ntff_profile_hook(hook)
        return hook is not None
    except Exception as e:  # degrade to untraced run
        print(f"ntff hook shim failed ({e}); running without trace")
        return False


def kernel(x, w1, b1, w2, b2):
    global _NC
    import os

    from concourse.bass_utils import run_bass_kernel_spmd

    shards, weights, slots = _host_prep(x, w1, b1, w2, b2)
    if _NC is None:
        _NC = _build_nc()

    in_maps = [{"x": shards[c], **weights} for c in range(NCORES)]
    trace = os.environ.get("AFNO_TRACE", "0") == "1"
    if trace:
        trace = _ensure_ntff_hook()
    res = run_bass_kernel_spmd(_NC, in_maps, core_ids=list(range(NCORES)), trace=trace)
    if trace and res.exec_time_ns is not None:
        print(f"HW exec time: {res.exec_time_ns} ns")
        if res.instructions_and_trace is not None:
            print(f"trace: {res.instructions_and_trace[1]}")

    x = np.asarray(x, dtype=np.float32).reshape(H, W, C)
    out = np.empty((H, W, C), dtype=np.float32)
    for c in range(NCORES):
        ro = res.results[c]["out"]
        for si in range(NSLOT):
            slot = slots[c * NSLOT + si]
            if slot is None:
                continue
            a, b = slot
            out[a] = ro[2 * si] + x[a]
            if b != a:
                out[b] = _revw(ro[2 * si + 1]) + x[b]
    return out.reshape(1, H, W, C)

